# revision 7
# baseline (speedup 1.0000x reference)
"""Mamba block (MockMambaBlock) on 8 Trainium2 NeuronCores.

Sharding: tensor-parallel over d_inner (8 x 256 channels), both batches on
every core. The x_proj/dt_proj contraction over d_inner is completed with an
on-device AllReduce of the small (32, L) partial per batch; out_proj
row-partials are summed on the host (the gather step).

Schedule: phase A computes the in_proj x-half (conv + x_proj) of batch 0
first so its AllReduce fires early; the z-half and batch 1 follow. The SSM
scans (the DVE wall: 2 cycles/element regardless of dtype) start as soon as
md(b0) lands and overlap the rest of phase A. All u = dtx*B_n elementwise
multiplies run on the otherwise-idle GpSimd engine; exp/silu/sigmoid and the
PSUM->SBUF drains run on the scalar engine, interleaved so the scan supply
chain never starves. out_proj is emitted per 128-token tile right behind the
gating to keep the tail short. Issue order doubles as per-engine program
order, so hooks thread low-priority work into the busy streams.
"""

import sys

sys.path.insert(0, "/opt/trn_rl_repo")

import numpy as np
import ml_dtypes

import concourse.bass as bass
import concourse.bacc as bacc
import concourse.mybir as mybir
import concourse.tile as tile
from concourse.bass_utils import run_bass_kernel_spmd

F32 = mybir.dt.float32
BF16 = mybir.dt.bfloat16
AF = mybir.ActivationFunctionType
OP = mybir.AluOpType

B, L, DM, DI, DS, DC = 2, 2048, 1024, 2048, 16, 4
NCORES = 8
DIL = DI // NCORES          # 256 channels per core
NBLK = DIL // 128           # 2 partition blocks of channels
KBLK = DM // 128            # 8 contraction blocks for in_proj
LTA = 512                   # token chunk
NCH = L // LTA              # 4 chunks
NPT = L // 512              # psum tiles per row


def build_nc():
    nc = bacc.Bacc()

    x_t = nc.dram_tensor("x_t", [B, KBLK, 128, L], BF16, kind="ExternalInput")
    win_d = nc.dram_tensor("win", [DM, 2 * DIL], BF16, kind="ExternalInput")
    wout_d = nc.dram_tensor("wout", [DIL, DM], BF16, kind="ExternalInput")
    wx_d = nc.dram_tensor("wx", [DIL, 2 * DS], BF16, kind="ExternalInput")
    wdt_d = nc.dram_tensor("wdt", [DS, DIL], BF16, kind="ExternalInput")
    a_d = nc.dram_tensor("a", [DIL, DS], F32, kind="ExternalInput")
    convw_d = nc.dram_tensor("convw", [DIL, DC], F32, kind="ExternalInput")
    convb_d = nc.dram_tensor("convb", [DIL, 1], F32, kind="ExternalInput")
    dvec_d = nc.dram_tensor("dvec", [DIL, 1], F32, kind="ExternalInput")
    bdt_d = nc.dram_tensor("bdt", [DIL, 1], F32, kind="ExternalInput")
    identb_d = nc.dram_tensor("identb", [128, 128], BF16, kind="ExternalInput")
    diagd_d = nc.dram_tensor("diagd", [DIL, 128], BF16, kind="ExternalInput")
    out_d = nc.dram_tensor("out_p", [B, L, DM], F32, kind="ExternalOutput")

    with tile.TileContext(nc) as tc:
        with (
            tc.tile_pool(name="weights", bufs=1) as wp,
            tc.tile_pool(name="resident", bufs=1) as rp,
            tc.tile_pool(name="dram", bufs=1, space="DRAM") as dp,
        ):
            # ---- weights to SBUF ----
            win_sb = wp.tile([128, KBLK, 2 * DIL], BF16)
            nc.sync.dma_start(win_sb[:], win_d[:].rearrange("(k p) m -> p k m", p=128))
            wout_sb = wp.tile([128, NBLK, DM], BF16)
            nc.sync.dma_start(wout_sb[:], wout_d[:].rearrange("(k p) m -> p k m", p=128))
            wx_sb = wp.tile([128, NBLK, 2 * DS], BF16)
            nc.sync.dma_start(wx_sb[:], wx_d[:].rearrange("(k p) m -> p k m", p=128))
            wdt_sb = wp.tile([DS, DIL], BF16)
            nc.sync.dma_start(wdt_sb[:], wdt_d[:])
            a_sb = wp.tile([128, NBLK, DS], F32)
            nc.sync.dma_start(a_sb[:], a_d[:].rearrange("(k p) m -> p k m", p=128))
            convw_sb = wp.tile([128, NBLK, DC], F32)
            nc.sync.dma_start(convw_sb[:], convw_d[:].rearrange("(k p) m -> p k m", p=128))
            convb_sb = wp.tile([128, NBLK, 1], F32)
            nc.sync.dma_start(convb_sb[:], convb_d[:].rearrange("(k p) m -> p k m", p=128))
            dvec_sb = wp.tile([128, NBLK, 1], F32)
            nc.sync.dma_start(dvec_sb[:], dvec_d[:].rearrange("(k p) m -> p k m", p=128))
            bdt_sb = wp.tile([128, NBLK, 1], F32)
            nc.sync.dma_start(bdt_sb[:], bdt_d[:].rearrange("(k p) m -> p k m", p=128))
            identb_sb = wp.tile([128, 128], BF16)
            nc.sync.dma_start(identb_sb[:], identb_d[:])
            diagd_sb = wp.tile([128, NBLK, 128], BF16)
            nc.sync.dma_start(diagd_sb[:], diagd_d[:].rearrange("(k p) m -> p k m", p=128))

            # ---- resident activations ----
            xcv = [[rp.tile([128, L], BF16, name=f"xcv{b_}{k}", tag=f"xcv{b_}{k}")
                    for k in range(NBLK)] for b_ in range(B)]
            zac = [[rp.tile([128, L], BF16, name=f"zac{b_}{k}", tag=f"zac{b_}{k}")
                    for k in range(NBLK)] for b_ in range(B)]
            # md holds -dt; after the scans it is overwritten in place by the
            # gated ssm output (yin) to save SBUF.
            md = [[rp.tile([128, L], BF16, name=f"md{b_}{k}", tag=f"md{b_}{k}")
                   for k in range(NBLK)] for b_ in range(B)]
            dtin_sb = [rp.tile([DS, L], BF16, name=f"dtin{b_}", tag=f"dtin{b_}")
                       for b_ in range(B)]
            xs_sb = [rp.tile([2 * DS, L], BF16, name=f"xs{b_}", tag=f"xs{b_}")
                     for b_ in range(B)]
            xp = [rp.tile([128, LTA + DC - 1], BF16, name=f"xp{k}", tag=f"xp{k}")
                  for k in range(NBLK)]

            cc_in = [dp.tile([2 * DS, L], BF16, name=f"cc_in{b_}") for b_ in range(B)]
            cc_out = [dp.tile([2 * DS, L], BF16, addr_space="Shared",
                              name=f"cc_out{b_}") for b_ in range(B)]
            ccw_in = dp.tile([2, 16], BF16, name="ccw_in")
            ccw_out = dp.tile([2, 16], BF16, addr_space="Shared", name="ccw_out")

            # warm up the CC stream so the first real AllReduce doesn't pay
            # the first-trigger latency
            nc.gpsimd.collective_compute(
                "AllReduce", OP.add,
                ins=[ccw_in.opt()], outs=[ccw_out.opt()],
                replica_groups=[list(range(NCORES))])

            with (
                tc.tile_pool(name="yps", bufs=1, space="PSUM") as ypsp,
                tc.tile_pool(name="dtps", bufs=2, space="PSUM") as dtpsp,
                tc.tile_pool(name="pax", bufs=1) as pax,
                tc.tile_pool(name="paz", bufs=1) as paz,
                tc.tile_pool(name="pb", bufs=1) as pb,
            ):
                # ---------------- phase A helpers ----------------
                def x_branch(b_, paps, mid_hook=None):
                    for ch in range(NCH):
                        t0 = ch * LTA
                        xsx = pax.tile([128, KBLK, LTA], BF16, tag="xsx", bufs=2,
                                       name=f"xsx{b_}{ch}")
                        nc.sync.dma_start(
                            xsx[:], x_t[b_].transpose([1, 0, 2])[:, :, t0:t0 + LTA])
                        for blk in range(NBLK):
                            ps = paps.tile([128, LTA], F32, tag="ps_in",
                                           name=f"psx{b_}{ch}{blk}")
                            for kb in range(KBLK):
                                nc.tensor.matmul(
                                    ps[:],
                                    win_sb[:, kb, blk * 128:(blk + 1) * 128],
                                    xsx[:, kb, :],
                                    start=(kb == 0), stop=(kb == KBLK - 1))
                            # depthwise causal conv, 4 taps, bf16
                            if ch == 0:
                                nc.vector.memset(xp[blk][:, 0:DC - 1], 0.0)
                            else:
                                nc.vector.tensor_copy(
                                    xp[blk][:, 0:DC - 1],
                                    xp[blk][:, LTA:LTA + DC - 1])
                            nc.scalar.copy(xp[blk][:, DC - 1:LTA + DC - 1], ps[:])
                            c0 = pax.tile([128, LTA], BF16, tag="cv0", bufs=2,
                                          name=f"c0{b_}{ch}{blk}")
                            c1 = pax.tile([128, LTA], BF16, tag="cv1", bufs=2,
                                          name=f"c1{b_}{ch}{blk}")
                            c2 = pax.tile([128, LTA], BF16, tag="cv2", bufs=2,
                                          name=f"c2{b_}{ch}{blk}")
                            nc.vector.tensor_scalar_mul(
                                c0[:], xp[blk][:, 0:LTA], convw_sb[:, blk, 0:1])
                            nc.vector.tensor_scalar_mul(
                                c1[:], xp[blk][:, 1:1 + LTA], convw_sb[:, blk, 1:2])
                            nc.vector.tensor_add(c0[:], c0[:], c1[:])
                            nc.vector.tensor_scalar_mul(
                                c1[:], xp[blk][:, 2:2 + LTA], convw_sb[:, blk, 2:3])
                            nc.vector.tensor_scalar_mul(
                                c2[:], xp[blk][:, 3:3 + LTA], convw_sb[:, blk, 3:4])
                            nc.vector.tensor_add(c1[:], c1[:], c2[:])
                            nc.vector.tensor_add(c0[:], c0[:], c1[:])
                            nc.scalar.activation(
                                xcv[b_][blk][:, t0:t0 + LTA], c0[:],
                                AF.Silu, bias=convb_sb[:, blk, :])
                        # x_proj partial for this chunk (borrows a ps_in slot)
                        psx = paps.tile([128, LTA], F32, tag="ps_in",
                                        name=f"psxp{b_}{ch}")
                        for kb in range(NBLK):
                            nc.tensor.matmul(
                                psx[0:2 * DS, :], wx_sb[:, kb, :],
                                xcv[b_][kb][:, t0:t0 + LTA],
                                start=(kb == 0), stop=(kb == NBLK - 1))
                        nc.scalar.copy(xs_sb[b_][:, t0:t0 + LTA], psx[0:2 * DS, :])
                        if ch == 2 and mid_hook is not None:
                            mid_hook()

                def z_load(b_, ch):
                    t0 = ch * LTA
                    xsz = paz.tile([128, KBLK, LTA], BF16, tag="xsz", bufs=2,
                                   name=f"xsz{b_}{ch}")
                    nc.sync.dma_start(
                        xsz[:], x_t[b_].transpose([1, 0, 2])[:, :, t0:t0 + LTA])
                    return xsz

                def z_mms(b_, ch, xsz, paps):
                    outs = []
                    for blk in range(NBLK):
                        ps = paps.tile([128, LTA], F32, tag="ps_in",
                                       name=f"psz{b_}{ch}{blk}")
                        for kb in range(KBLK):
                            nc.tensor.matmul(
                                ps[:],
                                win_sb[:, kb, (NBLK + blk) * 128:(NBLK + blk + 1) * 128],
                                xsz[:, kb, :],
                                start=(kb == 0), stop=(kb == KBLK - 1))
                        outs.append((blk, ps))
                    return outs

                def z_silu(b_, ch, outs):
                    t0 = ch * LTA
                    for blk, ps in outs:
                        nc.scalar.activation(
                            zac[b_][blk][:, t0:t0 + LTA], ps[:], AF.Silu)

                def dt_half(b_, blk):
                    # md = -softplus(dt_raw + b_dt) = ln(sigmoid(-(dt_raw + b_dt)))
                    for ch in range(NCH):
                        t0 = ch * LTA
                        psd = dtpsp.tile([128, LTA], F32, tag="psd",
                                         name=f"psd{b_}{ch}{blk}")
                        nc.tensor.matmul(
                            psd[:], wdt_sb[:, blk * 128:(blk + 1) * 128],
                            dtin_sb[b_][:, t0:t0 + LTA],
                            start=True, stop=True)
                        nc.scalar.activation(
                            md[b_][blk][:, t0:t0 + LTA], psd[:],
                            AF.Sigmoid, bias=bdt_sb[:, blk, :], scale=-1.0)
                    nc.scalar.activation(md[b_][blk][:], md[b_][blk][:], AF.Ln)

                def issue_cc_dma(b_):
                    nc.sync.dma_start(cc_in[b_][:], xs_sb[b_][:])

                def issue_cc_trigger(b_):
                    nc.gpsimd.collective_compute(
                        "AllReduce", OP.add,
                        ins=[cc_in[b_].opt()], outs=[cc_out[b_].opt()],
                        replica_groups=[list(range(NCORES))])

                def issue_bb(b_, blk, n):
                    bb = pb.tile([128, L], BF16, tag="bbn", bufs=4,
                                 name=f"bb{b_}{blk}{n}")
                    nc.sync.dma_start(
                        bb[:],
                        cc_out[b_][DS + n:DS + n + 1, :].broadcast_to([128, L]))
                    return bb

                # ---------------- phase B helpers ----------------
                def ssm_block(b_, blk, pre_bb=(), scalar_hook=None, gp_hook=None):
                    """scans + y accumulation for one (batch, blk). Hooks let
                    us interleave other work into the scalar/gpsimd streams."""
                    dtx = pb.tile([128, L], BF16, tag="dtx", bufs=2,
                                  name=f"dtx{b_}{blk}")
                    nc.vector.tensor_mul(dtx[:], md[b_][blk][:], xcv[b_][blk][:])
                    y_ps = [ypsp.tile([128, 512], F32, tag=f"yps{pt}",
                                      name=f"yps{b_}{blk}{pt}")
                            for pt in range(NPT)]
                    for n in range(DS):
                        bb = pre_bb[n] if n < len(pre_bb) else issue_bb(b_, blk, n)
                        da = pb.tile([128, L], F32, tag="dan", bufs=3,
                                     name=f"da{b_}{blk}{n}")
                        nc.scalar.activation(da[:], md[b_][blk][:], AF.Exp,
                                             scale=a_sb[:, blk, n:n + 1])
                        if scalar_hook is not None:
                            scalar_hook(n)
                        u = pb.tile([128, L], BF16, tag="un", bufs=3,
                                    name=f"u{b_}{blk}{n}")
                        nc.gpsimd.tensor_mul(u[:], dtx[:], bb[:])
                        if gp_hook is not None:
                            gp_hook(n)
                        h = pb.tile([128, L], BF16, tag="hn", bufs=5,
                                    name=f"h{b_}{blk}{n}")
                        nc.vector.tensor_tensor_scan(h[:], da[:], u[:],
                                                     0.0, OP.mult, OP.add)
                        for pt in range(NPT):
                            nc.tensor.matmul(
                                y_ps[pt][:], identb_sb[:],
                                h[:, pt * 512:(pt + 1) * 512],
                                start=(n == 0), stop=False)
                    return y_ps

                def gate_block(b_, blk, y_ps, per_pt=None):
                    yin = md[b_][blk]
                    for pt in range(NPT):
                        nc.tensor.matmul(
                            y_ps[pt][:], diagd_sb[:, blk, :],
                            xcv[b_][blk][:, pt * 512:(pt + 1) * 512],
                            start=False, stop=True)
                        nc.vector.tensor_mul(
                            yin[:, pt * 512:(pt + 1) * 512], y_ps[pt][:],
                            zac[b_][blk][:, pt * 512:(pt + 1) * 512])
                        if per_pt is not None:
                            per_pt(pt)
                    return yin

                def out_proj_mt(b_, yins, mt, dmh, psop):
                    ps_o = psop.tile([128, 512], F32, tag="ps_o", bufs=2,
                                     name=f"pso{b_}{mt}{dmh}")
                    for blk in range(NBLK):
                        nc.tensor.matmul(
                            ps_o[:],
                            yins[blk][:, mt * 128:(mt + 1) * 128],
                            wout_sb[:, blk, dmh * 512:(dmh + 1) * 512],
                            start=(blk == 0), stop=(blk == NBLK - 1))
                    osb = pb.tile([128, 512], F32, tag="osb", bufs=2,
                                  name=f"osb{b_}{mt}{dmh}")
                    nc.scalar.copy(osb[:], ps_o[:])
                    nc.sync.dma_start(
                        out_d[b_, mt * 128:(mt + 1) * 128,
                              dmh * 512:(dmh + 1) * 512],
                        osb[:])

                # ================= issue: phase A =================
                with tc.tile_pool(name="paps", bufs=2, space="PSUM") as paps:
                    x_branch(0, paps)
                    z0_tiles = [z_load(0, ch) for ch in range(NCH)]
                    issue_cc_dma(0)
                    issue_cc_trigger(0)
                    for ch in range(NCH):
                        z_silu(0, ch, z_mms(0, ch, z0_tiles[ch], paps))

                    def mid_b1():
                        nc.sync.dma_start(dtin_sb[0][:], cc_out[0][0:DS, :])
                        dt_half(0, 0)
                        dt_half(0, 1)

                    x_branch(1, paps, mid_hook=mid_b1)
                    # early B_ssm broadcasts for (b0, blk0) so the sync queue
                    # serves the first scans before the b1 bulk DMAs
                    pre_bb00 = [issue_bb(0, 0, n) for n in range(4)]
                    z1_tiles = [z_load(1, ch) for ch in range(NCH)]
                    issue_cc_dma(1)
                    # the b1 collective trigger is threaded into the gpsimd
                    # stream between u-multiplies (gp_hook below): issuing it
                    # here would stall gpsimd until cc_in(b1) lands (~66us)
                    # while the first scans need their u products at ~55us.
                    z1_mm_out = [z_mms(1, ch, z1_tiles[ch], paps) for ch in range(NCH)]

                # ================= issue: phase B =================
                with tc.tile_pool(name="pso", bufs=1, space="PSUM") as psop:
                    # (b0, blk0): interleave z(b1) silus into the exp stream
                    def sc_hook_b0(n):
                        if n % 4 == 1 and n // 4 < NCH:
                            z_silu(1, n // 4, z1_mm_out[n // 4])

                    def gp_hook_b0(n):
                        if n == 8:
                            issue_cc_trigger(1)

                    yps00 = ssm_block(0, 0, pre_bb=pre_bb00,
                                      scalar_hook=sc_hook_b0,
                                      gp_hook=gp_hook_b0)

                    # dt(b1) blk0: tensor work lands after identity(b0,blk0)
                    nc.sync.dma_start(dtin_sb[1][:], cc_out[1][0:DS, :])
                    dt_half(1, 0)

                    yps01 = ssm_block(0, 1)
                    dt_half(1, 1)
                    yin00 = gate_block(0, 0, yps00)
                    yin01 = gate_block(0, 1, yps01)

                    # (b1, blk0): interleave out_proj(b0) behind the exps
                    op0 = [(mt, dmh) for mt in range(L // 128) for dmh in range(2)]

                    def op_hook_b1(n):
                        for j in range(2):
                            idx = n * 2 + j
                            if idx < len(op0):
                                mt, dmh = op0[idx]
                                out_proj_mt(0, [yin00, yin01], mt, dmh, psop)

                    yps10 = ssm_block(1, 0, gp_hook=op_hook_b1)
                    yps11 = ssm_block(1, 1)
                    yin10 = gate_block(1, 0, yps10)

                    # gate blk1 per-pt and chase it with out_proj(b1)
                    def op1_pt(pt):
                        for mt in range(pt * 4, pt * 4 + 4):
                            for dmh in range(2):
                                out_proj_mt(1, [yin10, md[1][1]], mt, dmh, psop)

                    gate_block(1, 1, yps11, per_pt=op1_pt)

    nc.compile()
    return nc


_NC_CACHE = {}


def _get_nc():
    if "nc" not in _NC_CACHE:
        _NC_CACHE["nc"] = build_nc()
    return _NC_CACHE["nc"]


def make_in_maps(x, W_in, conv_w, conv_b, W_x, W_dt, b_dt, A_log, D, W_out):
    x = np.asarray(x, np.float32)
    W_in = np.asarray(W_in, np.float32)
    conv_w = np.asarray(conv_w, np.float32)
    conv_b = np.asarray(conv_b, np.float32)
    W_x = np.asarray(W_x, np.float32)
    W_dt = np.asarray(W_dt, np.float32)
    b_dt = np.asarray(b_dt, np.float32)
    A_log = np.asarray(A_log, np.float32)
    D = np.asarray(D, np.float32)
    W_out = np.asarray(W_out, np.float32)

    xt = np.ascontiguousarray(x.transpose(0, 2, 1)).reshape(B, KBLK, 128, L).astype(ml_dtypes.bfloat16)
    A = np.exp(A_log)  # positive |A|; md = -dt on device

    in_maps = []
    for c in range(NCORES):
        lo = c * DIL
        sl = slice(lo, lo + DIL)
        in_maps.append({
            "x_t": xt,
            "win": np.ascontiguousarray(
                np.concatenate([W_in[:, sl], W_in[:, DI + lo:DI + lo + DIL]],
                               axis=1)).astype(ml_dtypes.bfloat16),
            "wout": np.ascontiguousarray(W_out[sl]).astype(ml_dtypes.bfloat16),
            "wx": np.ascontiguousarray(
                np.concatenate([W_x[sl, :DS], -W_x[sl, DS:]], axis=1)
            ).astype(ml_dtypes.bfloat16),
            "wdt": np.ascontiguousarray(W_dt[:, sl]).astype(ml_dtypes.bfloat16),
            "a": np.ascontiguousarray(A[sl]),
            "convw": np.ascontiguousarray(conv_w[sl]),
            "convb": np.ascontiguousarray(conv_b[sl, None]),
            "dvec": np.ascontiguousarray(D[sl, None]),
            "bdt": np.ascontiguousarray(-b_dt[sl, None]),
            "identb": np.eye(128, dtype=ml_dtypes.bfloat16),
            "diagd": np.stack([np.diag(D[lo + k * 128:lo + (k + 1) * 128])
                               for k in range(NBLK)]).reshape(DIL, 128)
                       .astype(ml_dtypes.bfloat16),
        })
    return in_maps


def kernel(**inputs):
    nc = _get_nc()
    in_maps = make_in_maps(**inputs)
    res = run_bass_kernel_spmd(nc, in_maps, list(range(NCORES)))
    out = np.zeros((B, L, DM), np.float32)
    for c in range(NCORES):
        out += res.results[c]["out_p"]
    return out


# revision 9
# speedup vs baseline: 1.2917x; 1.2917x over previous
"""Mamba block (MockMambaBlock) on 8 Trainium2 NeuronCores.

Sharding: tensor-parallel over d_inner (8 x 256 channels), both batches on
every core. The x_proj/dt_proj contraction over d_inner is completed with an
on-device AllReduce of the small (32, L) partial per batch; out_proj
row-partials are summed on the host (the gather step).

Schedule: phase A computes the in_proj x-half (conv + x_proj) of batch 0
first so its AllReduce fires early; the z-half and batch 1 follow. The SSM
scans (the DVE wall: 2 cycles/element regardless of dtype) start as soon as
md(b0) lands and overlap the rest of phase A. The u = dtx*B_n multiplies stay on
the DVE (GpSimd shares SBUF ports with the DVE, so offloading them slows the
scans more than it saves); exp/silu/sigmoid and the PSUM->SBUF drains run on
the scalar engine, interleaved so the scan supply chain never starves. out_proj is emitted per 128-token tile right behind the
gating to keep the tail short. Issue order doubles as per-engine program
order, so hooks thread low-priority work into the busy streams.
"""

import sys

sys.path.insert(0, "/opt/trn_rl_repo")

import numpy as np
import ml_dtypes

import concourse.bass as bass
import concourse.bacc as bacc
import concourse.mybir as mybir
import concourse.tile as tile
from concourse.bass_utils import run_bass_kernel_spmd

F32 = mybir.dt.float32
BF16 = mybir.dt.bfloat16
AF = mybir.ActivationFunctionType
OP = mybir.AluOpType

B, L, DM, DI, DS, DC = 2, 2048, 1024, 2048, 16, 4
NCORES = 8
DIL = DI // NCORES          # 256 channels per core
NBLK = DIL // 128           # 2 partition blocks of channels
KBLK = DM // 128            # 8 contraction blocks for in_proj
LTA = 512                   # token chunk
NCH = L // LTA              # 4 chunks
NPT = L // 512              # psum tiles per row


def build_nc():
    nc = bacc.Bacc()

    x_t = nc.dram_tensor("x_t", [B, KBLK, 128, L], BF16, kind="ExternalInput")
    win_d = nc.dram_tensor("win", [DM, 2 * DIL], BF16, kind="ExternalInput")
    wout_d = nc.dram_tensor("wout", [DIL, DM], BF16, kind="ExternalInput")
    wx_d = nc.dram_tensor("wx", [DIL, 2 * DS], BF16, kind="ExternalInput")
    wdt_d = nc.dram_tensor("wdt", [DS, DIL], BF16, kind="ExternalInput")
    a_d = nc.dram_tensor("a", [DIL, DS], F32, kind="ExternalInput")
    convw_d = nc.dram_tensor("convw", [DIL, DC], F32, kind="ExternalInput")
    convb_d = nc.dram_tensor("convb", [DIL, 1], F32, kind="ExternalInput")
    dvec_d = nc.dram_tensor("dvec", [DIL, 1], F32, kind="ExternalInput")
    bdt_d = nc.dram_tensor("bdt", [DIL, 1], F32, kind="ExternalInput")
    identb_d = nc.dram_tensor("identb", [128, 128], BF16, kind="ExternalInput")
    diagd_d = nc.dram_tensor("diagd", [DIL, 128], BF16, kind="ExternalInput")
    out_d = nc.dram_tensor("out_p", [B, L, DM], F32, kind="ExternalOutput")

    with tile.TileContext(nc) as tc:
        with (
            tc.tile_pool(name="weights", bufs=1) as wp,
            tc.tile_pool(name="resident", bufs=1) as rp,
            tc.tile_pool(name="dram", bufs=1, space="DRAM") as dp,
        ):
            # ---- weights to SBUF ----
            win_sb = wp.tile([128, KBLK, 2 * DIL], BF16)
            nc.sync.dma_start(win_sb[:], win_d[:].rearrange("(k p) m -> p k m", p=128))
            wout_sb = wp.tile([128, NBLK, DM], BF16)
            nc.sync.dma_start(wout_sb[:], wout_d[:].rearrange("(k p) m -> p k m", p=128))
            wx_sb = wp.tile([128, NBLK, 2 * DS], BF16)
            nc.sync.dma_start(wx_sb[:], wx_d[:].rearrange("(k p) m -> p k m", p=128))
            wdt_sb = wp.tile([DS, DIL], BF16)
            nc.sync.dma_start(wdt_sb[:], wdt_d[:])
            a_sb = wp.tile([128, NBLK, DS], F32)
            nc.sync.dma_start(a_sb[:], a_d[:].rearrange("(k p) m -> p k m", p=128))
            convw_sb = wp.tile([128, NBLK, DC], F32)
            nc.sync.dma_start(convw_sb[:], convw_d[:].rearrange("(k p) m -> p k m", p=128))
            convb_sb = wp.tile([128, NBLK, 1], F32)
            nc.sync.dma_start(convb_sb[:], convb_d[:].rearrange("(k p) m -> p k m", p=128))
            dvec_sb = wp.tile([128, NBLK, 1], F32)
            nc.sync.dma_start(dvec_sb[:], dvec_d[:].rearrange("(k p) m -> p k m", p=128))
            bdt_sb = wp.tile([128, NBLK, 1], F32)
            nc.sync.dma_start(bdt_sb[:], bdt_d[:].rearrange("(k p) m -> p k m", p=128))
            identb_sb = wp.tile([128, 128], BF16)
            nc.sync.dma_start(identb_sb[:], identb_d[:])
            diagd_sb = wp.tile([128, NBLK, 128], BF16)
            nc.sync.dma_start(diagd_sb[:], diagd_d[:].rearrange("(k p) m -> p k m", p=128))

            # ---- resident activations ----
            xcv = [[rp.tile([128, L], BF16, name=f"xcv{b_}{k}", tag=f"xcv{b_}{k}")
                    for k in range(NBLK)] for b_ in range(B)]
            zac = [[rp.tile([128, L], BF16, name=f"zac{b_}{k}", tag=f"zac{b_}{k}")
                    for k in range(NBLK)] for b_ in range(B)]
            # md holds -dt; after the scans it is overwritten in place by the
            # gated ssm output (yin) to save SBUF.
            md = [[rp.tile([128, L], BF16, name=f"md{b_}{k}", tag=f"md{b_}{k}")
                   for k in range(NBLK)] for b_ in range(B)]
            dtin_sb = [rp.tile([DS, L], BF16, name=f"dtin{b_}", tag=f"dtin{b_}")
                       for b_ in range(B)]
            xs_sb = [rp.tile([2 * DS, L], BF16, name=f"xs{b_}", tag=f"xs{b_}")
                     for b_ in range(B)]
            xp = [rp.tile([128, LTA + DC - 1], BF16, name=f"xp{k}", tag=f"xp{k}")
                  for k in range(NBLK)]

            cc_in = [dp.tile([2 * DS, L], BF16, name=f"cc_in{b_}") for b_ in range(B)]
            cc_out = [dp.tile([2 * DS, L], BF16, addr_space="Shared",
                              name=f"cc_out{b_}") for b_ in range(B)]

            with (
                tc.tile_pool(name="yps", bufs=1, space="PSUM") as ypsp,
                tc.tile_pool(name="dtps", bufs=2, space="PSUM") as dtpsp,
                tc.tile_pool(name="pax", bufs=1) as pax,
                tc.tile_pool(name="paz", bufs=1) as paz,
                tc.tile_pool(name="pb", bufs=1) as pb,
            ):
                # ---------------- phase A helpers ----------------
                def x_branch(b_, paps):
                    for ch in range(NCH):
                        t0 = ch * LTA
                        xsx = pax.tile([128, KBLK, LTA], BF16, tag="xsx", bufs=2,
                                       name=f"xsx{b_}{ch}")
                        nc.sync.dma_start(
                            xsx[:], x_t[b_].transpose([1, 0, 2])[:, :, t0:t0 + LTA])
                        for blk in range(NBLK):
                            ps = paps.tile([128, LTA], F32, tag="ps_in",
                                           name=f"psx{b_}{ch}{blk}")
                            for kb in range(KBLK):
                                nc.tensor.matmul(
                                    ps[:],
                                    win_sb[:, kb, blk * 128:(blk + 1) * 128],
                                    xsx[:, kb, :],
                                    start=(kb == 0), stop=(kb == KBLK - 1))
                            # depthwise causal conv, 4 taps, bf16
                            if ch == 0:
                                nc.vector.memset(xp[blk][:, 0:DC - 1], 0.0)
                            else:
                                nc.vector.tensor_copy(
                                    xp[blk][:, 0:DC - 1],
                                    xp[blk][:, LTA:LTA + DC - 1])
                            nc.scalar.copy(xp[blk][:, DC - 1:LTA + DC - 1], ps[:])
                            c0 = pax.tile([128, LTA], BF16, tag="cv0", bufs=2,
                                          name=f"c0{b_}{ch}{blk}")
                            c1 = pax.tile([128, LTA], BF16, tag="cv1", bufs=2,
                                          name=f"c1{b_}{ch}{blk}")
                            c2 = pax.tile([128, LTA], BF16, tag="cv2", bufs=2,
                                          name=f"c2{b_}{ch}{blk}")
                            nc.vector.tensor_scalar_mul(
                                c0[:], xp[blk][:, 0:LTA], convw_sb[:, blk, 0:1])
                            nc.vector.tensor_scalar_mul(
                                c1[:], xp[blk][:, 1:1 + LTA], convw_sb[:, blk, 1:2])
                            nc.vector.tensor_add(c0[:], c0[:], c1[:])
                            nc.vector.tensor_scalar_mul(
                                c1[:], xp[blk][:, 2:2 + LTA], convw_sb[:, blk, 2:3])
                            nc.vector.tensor_scalar_mul(
                                c2[:], xp[blk][:, 3:3 + LTA], convw_sb[:, blk, 3:4])
                            nc.vector.tensor_add(c1[:], c1[:], c2[:])
                            nc.vector.tensor_add(c0[:], c0[:], c1[:])
                            nc.scalar.activation(
                                xcv[b_][blk][:, t0:t0 + LTA], c0[:],
                                AF.Silu, bias=convb_sb[:, blk, :])
                        # x_proj partial for this chunk (borrows a ps_in slot)
                        psx = paps.tile([128, LTA], F32, tag="ps_in",
                                        name=f"psxp{b_}{ch}")
                        for kb in range(NBLK):
                            nc.tensor.matmul(
                                psx[0:2 * DS, :], wx_sb[:, kb, :],
                                xcv[b_][kb][:, t0:t0 + LTA],
                                start=(kb == 0), stop=(kb == NBLK - 1))
                        nc.scalar.copy(xs_sb[b_][:, t0:t0 + LTA], psx[0:2 * DS, :])

                def z_load(b_, ch):
                    t0 = ch * LTA
                    xsz = paz.tile([128, KBLK, LTA], BF16, tag="xsz", bufs=2,
                                   name=f"xsz{b_}{ch}")
                    nc.sync.dma_start(
                        xsz[:], x_t[b_].transpose([1, 0, 2])[:, :, t0:t0 + LTA])
                    return xsz

                def z_mms(b_, ch, xsz, paps):
                    outs = []
                    for blk in range(NBLK):
                        ps = paps.tile([128, LTA], F32, tag="ps_in",
                                       name=f"psz{b_}{ch}{blk}")
                        for kb in range(KBLK):
                            nc.tensor.matmul(
                                ps[:],
                                win_sb[:, kb, (NBLK + blk) * 128:(NBLK + blk + 1) * 128],
                                xsz[:, kb, :],
                                start=(kb == 0), stop=(kb == KBLK - 1))
                        outs.append((blk, ps))
                    return outs

                def z_silu(b_, ch, outs):
                    t0 = ch * LTA
                    for blk, ps in outs:
                        nc.scalar.activation(
                            zac[b_][blk][:, t0:t0 + LTA], ps[:], AF.Silu)

                def dt_half(b_, blk):
                    # md = -softplus(dt_raw + b_dt) = ln(sigmoid(-(dt_raw + b_dt)))
                    for ch in range(NCH):
                        t0 = ch * LTA
                        psd = dtpsp.tile([128, LTA], F32, tag="psd",
                                         name=f"psd{b_}{ch}{blk}")
                        nc.tensor.matmul(
                            psd[:], wdt_sb[:, blk * 128:(blk + 1) * 128],
                            dtin_sb[b_][:, t0:t0 + LTA],
                            start=True, stop=True)
                        nc.scalar.activation(
                            md[b_][blk][:, t0:t0 + LTA], psd[:],
                            AF.Sigmoid, bias=bdt_sb[:, blk, :], scale=-1.0)
                    nc.scalar.activation(md[b_][blk][:], md[b_][blk][:], AF.Ln)

                def issue_cc_dma(b_):
                    nc.sync.dma_start(cc_in[b_][:], xs_sb[b_][:])

                def issue_cc_trigger(b_):
                    nc.gpsimd.collective_compute(
                        "AllReduce", OP.add,
                        ins=[cc_in[b_].opt()], outs=[cc_out[b_].opt()],
                        replica_groups=[list(range(NCORES))])

                def issue_bb(b_, blk, n):
                    bb = pb.tile([128, L], BF16, tag="bbn", bufs=4,
                                 name=f"bb{b_}{blk}{n}")
                    nc.sync.dma_start(
                        bb[:],
                        cc_out[b_][DS + n:DS + n + 1, :].broadcast_to([128, L]))
                    return bb

                # ---------------- phase B helpers ----------------
                def ssm_block(b_, blk, pre_bb=(), scalar_hook=None):
                    """scans + y accumulation for one (batch, blk). Hooks let
                    us interleave other work into the scalar/gpsimd streams."""
                    dtx = pb.tile([128, L], BF16, tag="dtx", bufs=2,
                                  name=f"dtx{b_}{blk}")
                    nc.vector.tensor_mul(dtx[:], md[b_][blk][:], xcv[b_][blk][:])
                    y_ps = [ypsp.tile([128, 512], F32, tag=f"yps{pt}",
                                      name=f"yps{b_}{blk}{pt}")
                            for pt in range(NPT)]
                    for n in range(DS):
                        bb = pre_bb[n] if n < len(pre_bb) else issue_bb(b_, blk, n)
                        da = pb.tile([128, L], F32, tag="dan", bufs=3,
                                     name=f"da{b_}{blk}{n}")
                        nc.scalar.activation(da[:], md[b_][blk][:], AF.Exp,
                                             scale=a_sb[:, blk, n:n + 1])
                        if scalar_hook is not None:
                            scalar_hook(n)
                        u = pb.tile([128, L], BF16, tag="un", bufs=3,
                                    name=f"u{b_}{blk}{n}")
                        nc.vector.tensor_mul(u[:], dtx[:], bb[:])
                        h = pb.tile([128, L], BF16, tag="hn", bufs=5,
                                    name=f"h{b_}{blk}{n}")
                        nc.vector.tensor_tensor_scan(h[:], da[:], u[:],
                                                     0.0, OP.mult, OP.add)
                        for pt in range(NPT):
                            nc.tensor.matmul(
                                y_ps[pt][:], identb_sb[:],
                                h[:, pt * 512:(pt + 1) * 512],
                                start=(n == 0), stop=False)
                    return y_ps

                def gate_block(b_, blk, y_ps, per_pt=None):
                    yin = md[b_][blk]
                    for pt in range(NPT):
                        nc.tensor.matmul(
                            y_ps[pt][:], diagd_sb[:, blk, :],
                            xcv[b_][blk][:, pt * 512:(pt + 1) * 512],
                            start=False, stop=True)
                        nc.vector.tensor_mul(
                            yin[:, pt * 512:(pt + 1) * 512], y_ps[pt][:],
                            zac[b_][blk][:, pt * 512:(pt + 1) * 512])
                        if per_pt is not None:
                            per_pt(pt)
                    return yin

                def out_proj_mt(b_, yins, mt, dmh, psop):
                    ps_o = psop.tile([128, 512], F32, tag="ps_o", bufs=2,
                                     name=f"pso{b_}{mt}{dmh}")
                    for blk in range(NBLK):
                        nc.tensor.matmul(
                            ps_o[:],
                            yins[blk][:, mt * 128:(mt + 1) * 128],
                            wout_sb[:, blk, dmh * 512:(dmh + 1) * 512],
                            start=(blk == 0), stop=(blk == NBLK - 1))
                    osb = pb.tile([128, 512], F32, tag="osb", bufs=2,
                                  name=f"osb{b_}{mt}{dmh}")
                    nc.scalar.copy(osb[:], ps_o[:])
                    nc.sync.dma_start(
                        out_d[b_, mt * 128:(mt + 1) * 128,
                              dmh * 512:(dmh + 1) * 512],
                        osb[:])

                # ================= issue: phase A =================
                with tc.tile_pool(name="paps", bufs=2, space="PSUM") as paps:
                    x_branch(0, paps)
                    z0_tiles = [z_load(0, ch) for ch in range(NCH)]
                    issue_cc_dma(0)
                    issue_cc_trigger(0)
                    for ch in range(NCH):
                        z_silu(0, ch, z_mms(0, ch, z0_tiles[ch], paps))

                    x_branch(1, paps)
                    z1_tiles = [z_load(1, ch) for ch in range(NCH)]
                    issue_cc_dma(1)
                    issue_cc_trigger(1)
                    # early B_ssm broadcasts for (b0, blk0): issued before the
                    # dtin DMA below so the blocked sync queue doesn't delay
                    # the first scans' operands
                    pre_bb00 = [issue_bb(0, 0, n) for n in range(4)]
                    nc.sync.dma_start(dtin_sb[0][:], cc_out[0][0:DS, :])
                    for ch in range(NCH):
                        z_silu(1, ch, z_mms(1, ch, z1_tiles[ch], paps))
                    dt_half(0, 0)
                    dt_half(0, 1)

                # ================= issue: phase B =================
                with tc.tile_pool(name="pso", bufs=1, space="PSUM") as psop:
                    yps00 = ssm_block(0, 0, pre_bb=pre_bb00)

                    # dt(b1) blk0: tensor work lands after identity(b0,blk0)
                    nc.sync.dma_start(dtin_sb[1][:], cc_out[1][0:DS, :])
                    dt_half(1, 0)

                    yps01 = ssm_block(0, 1)
                    dt_half(1, 1)
                    yin00 = gate_block(0, 0, yps00)
                    yin01 = gate_block(0, 1, yps01)

                    # (b1, blk0): interleave out_proj(b0) behind the exps so
                    # the scalar engine drains PSUM without starving the scans
                    op0 = [(mt, dmh) for mt in range(L // 128) for dmh in range(2)]

                    def op_hook_b1(n):
                        for j in range(2):
                            idx = n * 2 + j
                            if idx < len(op0):
                                mt, dmh = op0[idx]
                                out_proj_mt(0, [yin00, yin01], mt, dmh, psop)

                    yps10 = ssm_block(1, 0, scalar_hook=op_hook_b1)
                    yps11 = ssm_block(1, 1)
                    yin10 = gate_block(1, 0, yps10)

                    # gate blk1 per-pt and chase it with out_proj(b1)
                    def op1_pt(pt):
                        for mt in range(pt * 4, pt * 4 + 4):
                            for dmh in range(2):
                                out_proj_mt(1, [yin10, md[1][1]], mt, dmh, psop)

                    gate_block(1, 1, yps11, per_pt=op1_pt)

    nc.compile()
    return nc


_NC_CACHE = {}


def _get_nc():
    if "nc" not in _NC_CACHE:
        _NC_CACHE["nc"] = build_nc()
    return _NC_CACHE["nc"]


def make_in_maps(x, W_in, conv_w, conv_b, W_x, W_dt, b_dt, A_log, D, W_out):
    x = np.asarray(x, np.float32)
    W_in = np.asarray(W_in, np.float32)
    conv_w = np.asarray(conv_w, np.float32)
    conv_b = np.asarray(conv_b, np.float32)
    W_x = np.asarray(W_x, np.float32)
    W_dt = np.asarray(W_dt, np.float32)
    b_dt = np.asarray(b_dt, np.float32)
    A_log = np.asarray(A_log, np.float32)
    D = np.asarray(D, np.float32)
    W_out = np.asarray(W_out, np.float32)

    xt = np.ascontiguousarray(x.transpose(0, 2, 1)).reshape(B, KBLK, 128, L).astype(ml_dtypes.bfloat16)
    A = np.exp(A_log)  # positive |A|; md = -dt on device

    in_maps = []
    for c in range(NCORES):
        lo = c * DIL
        sl = slice(lo, lo + DIL)
        in_maps.append({
            "x_t": xt,
            "win": np.ascontiguousarray(
                np.concatenate([W_in[:, sl], W_in[:, DI + lo:DI + lo + DIL]],
                               axis=1)).astype(ml_dtypes.bfloat16),
            "wout": np.ascontiguousarray(W_out[sl]).astype(ml_dtypes.bfloat16),
            "wx": np.ascontiguousarray(
                np.concatenate([W_x[sl, :DS], -W_x[sl, DS:]], axis=1)
            ).astype(ml_dtypes.bfloat16),
            "wdt": np.ascontiguousarray(W_dt[:, sl]).astype(ml_dtypes.bfloat16),
            "a": np.ascontiguousarray(A[sl]),
            "convw": np.ascontiguousarray(conv_w[sl]),
            "convb": np.ascontiguousarray(conv_b[sl, None]),
            "dvec": np.ascontiguousarray(D[sl, None]),
            "bdt": np.ascontiguousarray(-b_dt[sl, None]),
            "identb": np.eye(128, dtype=ml_dtypes.bfloat16),
            "diagd": np.stack([np.diag(D[lo + k * 128:lo + (k + 1) * 128])
                               for k in range(NBLK)]).reshape(DIL, 128)
                       .astype(ml_dtypes.bfloat16),
        })
    return in_maps


def kernel(**inputs):
    nc = _get_nc()
    in_maps = make_in_maps(**inputs)
    res = run_bass_kernel_spmd(nc, in_maps, list(range(NCORES)))
    out = np.zeros((B, L, DM), np.float32)
    for c in range(NCORES):
        out += res.results[c]["out_p"]
    return out
